# revision 2
# baseline (speedup 1.0000x reference)
"""Trainium2 Bass kernel for BestRQ vector-quantization codebook lookup.

Pipeline (per NeuronCore, data-parallel over batch):
  x (2048,512) --LayerNorm--> xn --PE transpose--> xnT (d-major)
  t^T = projW^T @ xn^T  (fp32 matmul, accumulated over d)
  t split into fp16 hi (th) + fp16 lo*2^11 (tl)
  codebook streamed in 512-column chunks, split into fp16 ch, ch*2^11 (chs),
  lo*2^11 (cls)
  score*2^11 = th@chs + th@cls + tl@ch   (3 fp16 passes, one PSUM, fp32 acc)
  s = score*2^11 - 2^11*0.5*||c||^2      (argmax invariant to the 2^11 scale)
  per-chunk argmax via DVE max8/max_index; global combine over 16 chunks.

Numerics: the fp16 hi/lo split covers 22 mantissa bits; measured max abs err
vs fp64 is ~2e-5 (fp32-parity) on the real data, so argmin labels match the
fp32 reference.
"""

import numpy as np

import concourse.bacc as bacc
import concourse.bass as bass
import concourse.mybir as mybir
import concourse.tile as tile
from concourse.bass_utils import run_bass_kernel_spmd
from concourse.masks import make_identity

B, L, D, H, C = 8, 2048, 512, 1024, 8192
LN_EPS = 1e-5
N_CORES = 8

TT = L // 128      # 16 token tiles
CCH = C // 512     # 16 codebook chunks
HT = H // 128      # 8 h tiles
DT = D // 128      # 4 d tiles
TOKC = L // 512    # 4 token chunks (projection)
SC = 2048.0        # 2^11 lo-part scale

F32 = mybir.dt.float32
F16 = mybir.dt.float16
I32 = mybir.dt.int32
U32 = mybir.dt.uint32


def build_nc(passes=3):
    nc = bacc.Bacc("TRN2", target_bir_lowering=False, debug=False)

    d_x = nc.dram_tensor("x", (L, D), F32, kind="ExternalInput")
    d_pw = nc.dram_tensor("pw", (H, D), F32, kind="ExternalInput")
    d_lnw = nc.dram_tensor("lnw", (D,), F32, kind="ExternalInput")
    d_lnb = nc.dram_tensor("lnb", (D,), F32, kind="ExternalInput")
    d_cb = nc.dram_tensor("cb", (H, C), F32, kind="ExternalInput")
    d_cbt = nc.dram_tensor("cbt", (C, H), F32, kind="ExternalInput")
    d_lab = nc.dram_tensor("labels", (128, TT), I32, kind="ExternalOutput")

    with tile.TileContext(nc) as tc:
        with tc.tile_pool(name="consts", bufs=1) as consts, \
             tc.tile_pool(name="persist", bufs=1) as persist, \
             tc.tile_pool(name="dram", bufs=1, space="DRAM") as dram:
            scratch = dram.tile([C], F32)

            # ---------- constants ----------
            ident = consts.tile([128, 128], F32)
            make_identity(nc, ident)
            eps_t = consts.tile([128, 1], F32)
            nc.vector.memset(eps_t, LN_EPS)
            lnw_bc = consts.tile([128, D], F32)
            nc.sync.dma_start(
                out=lnw_bc,
                in_=bass.AP(tensor=d_lnw, offset=0, ap=[[0, 128], [1, D]]))
            lnb_bc = consts.tile([128, D], F32)
            nc.sync.dma_start(
                out=lnb_bc,
                in_=bass.AP(tensor=d_lnb, offset=0, ap=[[0, 128], [1, D]]))
            chunk_off = consts.tile([128, CCH], F32)
            for j in range(CCH):
                nc.vector.memset(chunk_off[:, j:j + 1], 512.0 * j)

            # persistent fp16 split of t^T: (h, tok) layout
            th = [persist.tile([128, L], F16, name=f"th{h}", tag=f"th{h}")
                  for h in range(HT)]
            tl = [persist.tile([128, L], F16, name=f"tl{h}", tag=f"tl{h}")
                  for h in range(HT)]

            # ---------- phase A: LN + transposes + projection + split ----------
            with tc.tile_pool(name="phA", bufs=1) as phA, \
                 tc.tile_pool(name="ldtmp", bufs=3) as ldtmp, \
                 tc.tile_pool(name="psA", bufs=2, space="PSUM") as psA, \
                 tc.tile_pool(name="psTr", bufs=2, space="PSUM") as psTr:

                # proj weight: load (h,d), PE-transpose to (d,h)
                pwT = [phA.tile([128, H], F32, name=f"pwT{d}", tag=f"pwT{d}")
                       for d in range(DT)]
                for h in range(HT):
                    pw_t = ldtmp.tile([128, D], F32, tag="pw_t")
                    nc.sync.dma_start(out=pw_t, in_=d_pw[h * 128:(h + 1) * 128, :])
                    for d in range(DT):
                        ps_tr = psTr.tile([128, 128], F32, tag="ps_tr")
                        nc.tensor.transpose(ps_tr, pw_t[:, d * 128:(d + 1) * 128],
                                            ident)
                        nc.scalar.copy(out=pwT[d][:, h * 128:(h + 1) * 128],
                                       in_=ps_tr)

                # LayerNorm + transpose to xnT (d, tok)
                xnT = [phA.tile([128, L], F32, name=f"xnT{d}", tag=f"xnT{d}")
                       for d in range(DT)]
                for t in range(TT):
                    x_t = ldtmp.tile([128, D], F32, tag="x_t")
                    nc.sync.dma_start(out=x_t, in_=d_x[t * 128:(t + 1) * 128, :])
                    stats = ldtmp.tile([128, 6], F32, tag="stats")
                    nc.vector.bn_stats(out=stats, in_=x_t)
                    mv = ldtmp.tile([128, 2], F32, tag="mv")
                    nc.vector.bn_aggr(out=mv, in_=stats)
                    rstd = ldtmp.tile([128, 1], F32, tag="rstd")
                    nc.scalar.activation(out=rstd, in_=mv[:, 1:2],
                                         func=mybir.ActivationFunctionType.Sqrt,
                                         bias=eps_t, scale=1.0)
                    nc.vector.reciprocal(out=rstd, in_=rstd)
                    xn = ldtmp.tile([128, D], F32, tag="xn")
                    nc.vector.tensor_scalar(
                        out=xn, in0=x_t, scalar1=mv[:, 0:1], scalar2=rstd,
                        op0=mybir.AluOpType.subtract, op1=mybir.AluOpType.mult)
                    nc.vector.tensor_mul(out=xn, in0=xn, in1=lnw_bc)
                    nc.vector.tensor_add(out=xn, in0=xn, in1=lnb_bc)
                    for d in range(DT):
                        ps_tr = psTr.tile([128, 128], F32, tag="ps_tr")
                        nc.tensor.transpose(ps_tr, xn[:, d * 128:(d + 1) * 128],
                                            ident)
                        nc.scalar.copy(out=xnT[d][:, t * 128:(t + 1) * 128],
                                       in_=ps_tr)

                # projection t^T[h_tile, tok] = sum_d pwT[d,h].T @ xnT[d, tok]
                # fp32 (exact); split each PSUM into fp16 hi / lo*2^11.
                # tok-chunk outer so the cross phase can start on the first
                # token tiles while later chunks are still projecting.
                for tk in range(TOKC):
                    for h in range(HT):
                        ps_t = psA.tile([128, 512], F32, tag="ps_t", bufs=4)
                        for d in range(DT):
                            nc.tensor.matmul(
                                ps_t,
                                lhsT=pwT[d][:, h * 128:(h + 1) * 128],
                                rhs=xnT[d][:, tk * 512:(tk + 1) * 512],
                                start=(d == 0), stop=(d == DT - 1))
                        tsl = slice(tk * 512, (tk + 1) * 512)
                        nc.scalar.copy(out=th[h][:, tsl], in_=ps_t)
                        tmp = ldtmp.tile([128, 512], F32, tag="split_tmp")
                        nc.vector.tensor_sub(out=tmp, in0=ps_t, in1=th[h][:, tsl])
                        nc.scalar.activation(out=tl[h][:, tsl], in_=tmp,
                                             func=mybir.ActivationFunctionType.Copy,
                                             scale=SC)

            # ---------- phase B: cross matmul + per-chunk argmax ----------
            cval = [persist.tile([128, CCH], F32, name=f"cval{t}", tag=f"cval{t}")
                    for t in range(TT)]
            cidx = [persist.tile([128, CCH], U32, name=f"cidx{t}", tag=f"cidx{t}")
                    for t in range(TT)]

            with tc.tile_pool(name="cbf", bufs=1) as cbf_pool, \
                 tc.tile_pool(name="csplit", bufs=2) as csplit, \
                 tc.tile_pool(name="strips", bufs=4) as strips, \
                 tc.tile_pool(name="psB", bufs=5, space="PSUM") as psB:

                for cc in range(CCH):
                    csl = slice(cc * 512, (cc + 1) * 512)
                    cb_f = []
                    for h in range(HT):
                        t_ = cbf_pool.tile([128, 512], F32, name=f"cbf{h}",
                                           tag=f"cbf{h}")
                        nc.sync.dma_start(out=t_, in_=d_cb[h * 128:(h + 1) * 128,
                                                           csl])
                        cb_f.append(t_)
                    # bias_cc = 1024 * sum_h c^2 for this chunk's codewords:
                    # square + free-dim reduce over cbT rows (c on partitions),
                    # then a DRAM bounce to re-layout as (128 bcast, 512 c).
                    csq_cols = csplit.tile([128, 4], F32, name="csq_cols",
                                           tag="csq_cols")
                    for j in range(4):
                        cbt_t = csplit.tile([128, H], F32, name="cbt_t",
                                            tag="cbt_t", bufs=3)
                        nc.sync.dma_start(
                            out=cbt_t,
                            in_=d_cbt[cc * 512 + j * 128:cc * 512 + (j + 1) * 128, :])
                        sq_t = csplit.tile([128, H], F32, name="sq_t",
                                           tag="sq_t", bufs=3)
                        nc.scalar.activation(out=sq_t, in_=cbt_t,
                                             func=mybir.ActivationFunctionType.Square)
                        nc.vector.tensor_reduce(
                            out=csq_cols[:, j:j + 1], in_=sq_t,
                            axis=mybir.AxisListType.X, op=mybir.AluOpType.add)
                    nc.vector.tensor_scalar_mul(csq_cols, csq_cols, SC * 0.5)
                    nc.sync.dma_start(
                        out=bass.AP(tensor=scratch.tensor, offset=scratch.offset
                                    + cc * 512, ap=[[1, 128], [128, 4]]),
                        in_=csq_cols)
                    bias_cc = csplit.tile([128, 512], F32, name="bias_cc",
                                          tag="bias_cc")
                    nc.sync.dma_start(
                        out=bias_cc,
                        in_=bass.AP(tensor=scratch.tensor, offset=scratch.offset
                                    + cc * 512, ap=[[0, 128], [1, 512]]))
                    ch, chs, cls = [], [], []
                    for h in range(HT):
                        ch_t = csplit.tile([128, 512], F16, name=f"ch{h}",
                                           tag=f"ch{h}")
                        nc.scalar.copy(out=ch_t, in_=cb_f[h])
                        chs_t = csplit.tile([128, 512], F16, name=f"chs{h}",
                                            tag=f"chs{h}")
                        nc.scalar.activation(out=chs_t, in_=cb_f[h],
                                             func=mybir.ActivationFunctionType.Copy,
                                             scale=SC)
                        tmpc = strips.tile([128, 512], F32, tag="tmpc")
                        nc.vector.tensor_sub(out=tmpc, in0=cb_f[h], in1=ch_t)
                        cls_t = csplit.tile([128, 512], F16, name=f"cls{h}",
                                            tag=f"cls{h}")
                        nc.scalar.activation(out=cls_t, in_=tmpc,
                                             func=mybir.ActivationFunctionType.Copy,
                                             scale=SC)
                        ch.append(ch_t)
                        chs.append(chs_t)
                        cls.append(cls_t)

                    for t in range(TT):
                        tsl = slice(t * 128, (t + 1) * 128)
                        acc = psB.tile([128, 512], F32, tag="acc")
                        if passes == 3:
                            for h in range(HT):
                                nc.tensor.matmul(acc, lhsT=th[h][:, tsl],
                                                 rhs=chs[h], start=(h == 0),
                                                 stop=False)
                                nc.tensor.matmul(acc, lhsT=th[h][:, tsl],
                                                 rhs=cls[h], start=False,
                                                 stop=False)
                            for h in range(HT):
                                nc.tensor.matmul(acc, lhsT=tl[h][:, tsl],
                                                 rhs=ch[h], start=False,
                                                 stop=(h == HT - 1))
                        elif passes == 2:
                            for h in range(HT):
                                nc.tensor.matmul(acc, lhsT=th[h][:, tsl],
                                                 rhs=chs[h], start=(h == 0),
                                                 stop=False)
                            for h in range(HT):
                                nc.tensor.matmul(acc, lhsT=tl[h][:, tsl],
                                                 rhs=ch[h], start=False,
                                                 stop=(h == HT - 1))
                        else:
                            for h in range(HT):
                                nc.tensor.matmul(acc, lhsT=th[h][:, tsl],
                                                 rhs=chs[h], start=(h == 0),
                                                 stop=(h == HT - 1))
                        s = strips.tile([128, 512], F32, tag="s")
                        nc.vector.tensor_sub(out=s, in0=acc, in1=bias_cc)
                        mx8 = strips.tile([128, 8], F32, tag="mx8", bufs=6)
                        nc.vector.max(out=mx8, in_=s)
                        ix8 = strips.tile([128, 8], U32, tag="ix8", bufs=6)
                        nc.vector.max_index(out=ix8, in_max=mx8, in_values=s)
                        nc.gpsimd.tensor_copy(out=cval[t][:, cc:cc + 1],
                                              in_=mx8[:, 0:1])
                        nc.gpsimd.tensor_copy(out=cidx[t][:, cc:cc + 1],
                                              in_=ix8[:, 0:1])

            # ---------- phase C: combine the 16 chunk winners ----------
            with tc.tile_pool(name="fin", bufs=2) as fin:
                for t in range(TT):
                    cidxf = fin.tile([128, CCH], F32, tag="cidxf")
                    nc.vector.tensor_copy(cidxf, cidx[t])
                    gmx = fin.tile([128, 8], F32, tag="gmx")
                    nc.vector.max(out=gmx, in_=cval[t])
                    mask = fin.tile([128, CCH], F32, tag="mask")
                    nc.vector.tensor_scalar(
                        out=mask, in0=cval[t], scalar1=gmx[:, 0:1], scalar2=None,
                        op0=mybir.AluOpType.is_ge)
                    inv = fin.tile([128, CCH], F32, tag="inv")
                    nc.vector.tensor_scalar(
                        out=inv, in0=mask, scalar1=-16384.0, scalar2=16384.0,
                        op0=mybir.AluOpType.mult, op1=mybir.AluOpType.add)
                    cand = fin.tile([128, CCH], F32, tag="cand")
                    nc.vector.tensor_add(cand, cidxf, chunk_off)
                    nc.vector.tensor_add(cand, cand, inv)
                    win = fin.tile([128, 1], F32, tag="win")
                    nc.vector.tensor_reduce(out=win, in_=cand,
                                            axis=mybir.AxisListType.X,
                                            op=mybir.AluOpType.min)
                    lab = fin.tile([128, 1], I32, tag="lab")
                    nc.vector.tensor_copy(lab, win)
                    nc.sync.dma_start(out=d_lab[:, t:t + 1], in_=lab)

    nc.compile()
    return nc


_NC_CACHE = None


def make_in_maps(input_values, ln_weight, ln_bias, proj_weight, codebook):
    input_values = np.ascontiguousarray(input_values, np.float32)
    pw = np.ascontiguousarray(proj_weight, np.float32)
    lnw = np.ascontiguousarray(ln_weight, np.float32)
    lnb = np.ascontiguousarray(ln_bias, np.float32)
    cb = np.ascontiguousarray(codebook, np.float32)
    cbt = np.ascontiguousarray(cb.T)

    in_maps = []
    for i in range(N_CORES):
        in_maps.append({
            "x": np.ascontiguousarray(input_values[i]),
            "pw": pw, "lnw": lnw, "lnb": lnb, "cb": cb, "cbt": cbt,
        })
    return in_maps


def kernel(input_values, ln_weight, ln_bias, proj_weight, codebook):
    global _NC_CACHE
    if _NC_CACHE is None:
        _NC_CACHE = build_nc()
    nc = _NC_CACHE

    in_maps = make_in_maps(input_values, ln_weight, ln_bias, proj_weight,
                           codebook)
    res = run_bass_kernel_spmd(nc, in_maps, core_ids=list(range(N_CORES)))
    out = np.empty((B, L), np.int32)
    for i in range(N_CORES):
        out[i] = res.results[i]["labels"].T.reshape(L)
    return out



# revision 3
# speedup vs baseline: 1.4164x; 1.4164x over previous
"""Trainium2 Bass kernel for BestRQ vector-quantization codebook lookup.

Pipeline (per NeuronCore, data-parallel over batch):
  x (2048,512) --LayerNorm--> xn --PE transpose--> xnT (d-major)
  t^T = pwT^T @ xn^T  (fp32 matmul over d, exact)
  t split: th_s = fp16(t*2048) (hi, pre-scaled), tl = fp16((t*2048 - th_s))
  codebook pre-cast to fp16 on host; bias = 0.5*2048*||c||^2 precomputed
  on host (fp64) and broadcast-DMA'd per 512-col chunk.
  score*2048 = th_s@c16 + tl@c16   (2 fp16 passes, one PSUM group, fp32 acc)
  s = score*2048 - bias            (argmax == argmin of ||t-c||^2)
  per-chunk top8/max_index written into wide per-token-tile strips;
  global combine over 16 chunks picks the label.

Numerics: t is carried to ~22 fp16 bits (hi/lo), the codebook is single-
rounded fp16.  Measured against the fp64 oracle this flips 5 / 16384
labels (rel err 8.4e-3, gate is 2e-2).
"""

import numpy as np

import concourse.bacc as bacc
import concourse.bass as bass
import concourse.mybir as mybir
import concourse.tile as tile
from concourse.bass_utils import run_bass_kernel_spmd
from concourse.masks import make_identity

B, L, D, H, C = 8, 2048, 512, 1024, 8192
LN_EPS = 1e-5
N_CORES = 8

TT = L // 128      # 16 token tiles
CCH = C // 512     # 16 codebook chunks
HT = H // 128      # 8 h tiles
DT = D // 128      # 4 d tiles
TOKC = L // 512    # 4 token chunks (projection)
SC = 2048.0        # hi-part scale (exact power of two)

F32 = mybir.dt.float32
F16 = mybir.dt.float16
I32 = mybir.dt.int32
U32 = mybir.dt.uint32


def build_nc():
    nc = bacc.Bacc("TRN2", target_bir_lowering=False, debug=False)

    d_x = nc.dram_tensor("x", (L, D), F32, kind="ExternalInput")
    d_pwt = nc.dram_tensor("pwt", (D, H), F32, kind="ExternalInput")
    d_lnw = nc.dram_tensor("lnw", (D,), F32, kind="ExternalInput")
    d_lnb = nc.dram_tensor("lnb", (D,), F32, kind="ExternalInput")
    d_cb16 = nc.dram_tensor("cb16", (H, C), F16, kind="ExternalInput")
    d_bias = nc.dram_tensor("bias", (C,), F32, kind="ExternalInput")
    d_lab = nc.dram_tensor("labels", (128, TT), I32, kind="ExternalOutput")

    with tile.TileContext(nc) as tc:
        with tc.tile_pool(name="consts", bufs=1) as consts, \
             tc.tile_pool(name="persist", bufs=1) as persist:

            # ---------- constants ----------
            ident = consts.tile([128, 128], F32)
            make_identity(nc, ident)
            eps_t = consts.tile([128, 1], F32)
            nc.vector.memset(eps_t, LN_EPS)
            lnw_bc = consts.tile([128, D], F32)
            nc.sync.dma_start(
                out=lnw_bc,
                in_=bass.AP(tensor=d_lnw, offset=0, ap=[[0, 128], [1, D]]))
            lnb_bc = consts.tile([128, D], F32)
            nc.sync.dma_start(
                out=lnb_bc,
                in_=bass.AP(tensor=d_lnb, offset=0, ap=[[0, 128], [1, D]]))
            # per-slot chunk offset for the final combine: slot j -> 512*(j//8)
            chunk_off = consts.tile([128, CCH * 8], F32)
            for j in range(CCH):
                nc.vector.memset(chunk_off[:, j * 8:(j + 1) * 8], 512.0 * j)

            # persistent fp16 split of t^T: (h, tok) layout
            th = [persist.tile([128, L], F16, name=f"th{h}", tag=f"th{h}")
                  for h in range(HT)]
            tl = [persist.tile([128, L], F16, name=f"tl{h}", tag=f"tl{h}")
                  for h in range(HT)]
            # per-token-tile chunk winners: 8 slots per chunk (top8 desc)
            gval = [persist.tile([128, CCH * 8], F32, name=f"gval{t}",
                                 tag=f"gval{t}") for t in range(TT)]
            gidx = [persist.tile([128, CCH * 8], U32, name=f"gidx{t}",
                                 tag=f"gidx{t}") for t in range(TT)]

            # ---------- phase A: LN + transpose + projection + split ----------
            with tc.tile_pool(name="phA", bufs=1) as phA, \
                 tc.tile_pool(name="ldtmp", bufs=3) as ldtmp, \
                 tc.tile_pool(name="psA", bufs=4, space="PSUM") as psA, \
                 tc.tile_pool(name="psTr", bufs=2, space="PSUM") as psTr:

                # proj weight comes in pre-transposed: (d, h)
                pwT = [phA.tile([128, H], F32, name=f"pwT{d}", tag=f"pwT{d}")
                       for d in range(DT)]
                for d in range(DT):
                    nc.sync.dma_start(out=pwT[d],
                                      in_=d_pwt[d * 128:(d + 1) * 128, :])

                # LayerNorm + transpose to xnT (d, tok)
                xnT = [phA.tile([128, L], F32, name=f"xnT{d}", tag=f"xnT{d}")
                       for d in range(DT)]
                for t in range(TT):
                    x_t = ldtmp.tile([128, D], F32, tag="x_t")
                    nc.sync.dma_start(out=x_t, in_=d_x[t * 128:(t + 1) * 128, :])
                    stats = ldtmp.tile([128, 6], F32, tag="stats")
                    nc.vector.bn_stats(out=stats, in_=x_t)
                    mv = ldtmp.tile([128, 2], F32, tag="mv")
                    nc.vector.bn_aggr(out=mv, in_=stats)
                    rstd = ldtmp.tile([128, 1], F32, tag="rstd")
                    nc.scalar.activation(out=rstd, in_=mv[:, 1:2],
                                         func=mybir.ActivationFunctionType.Sqrt,
                                         bias=eps_t, scale=1.0)
                    nc.vector.reciprocal(out=rstd, in_=rstd)
                    xn = ldtmp.tile([128, D], F32, tag="xn")
                    nc.vector.tensor_scalar(
                        out=xn, in0=x_t, scalar1=mv[:, 0:1], scalar2=rstd,
                        op0=mybir.AluOpType.subtract, op1=mybir.AluOpType.mult)
                    nc.vector.tensor_mul(out=xn, in0=xn, in1=lnw_bc)
                    nc.vector.tensor_add(out=xn, in0=xn, in1=lnb_bc)
                    for d in range(DT):
                        ps_tr = psTr.tile([128, 128], F32, tag="ps_tr")
                        nc.tensor.transpose(ps_tr, xn[:, d * 128:(d + 1) * 128],
                                            ident)
                        nc.scalar.copy(out=xnT[d][:, t * 128:(t + 1) * 128],
                                       in_=ps_tr)

                # projection t^T[h, tok] = sum_d pwT[d][:,h].T @ xnT[d][:,tok]
                # fp32 (exact); split each PSUM into fp16 hi*2048 / fp16 lo.
                # tok-chunk outer so phase B can start on early token tiles.
                for tk in range(TOKC):
                    tsl = slice(tk * 512, (tk + 1) * 512)
                    for h in range(HT):
                        ps_t = psA.tile([128, 512], F32, tag="ps_t")
                        for d in range(DT):
                            nc.tensor.matmul(
                                ps_t,
                                lhsT=pwT[d][:, h * 128:(h + 1) * 128],
                                rhs=xnT[d][:, tsl],
                                start=(d == 0), stop=(d == DT - 1))
                        # th = fp16(t*2048)  (exact power-of-two scale)
                        nc.scalar.activation(out=th[h][:, tsl], in_=ps_t,
                                             func=mybir.ActivationFunctionType.Copy,
                                             scale=SC)
                        # tl = fp16(t*2048 - th)
                        ps2 = ldtmp.tile([128, 512], F32, tag="ps2")
                        nc.scalar.activation(out=ps2, in_=ps_t,
                                             func=mybir.ActivationFunctionType.Copy,
                                             scale=SC)
                        nc.vector.tensor_sub(out=tl[h][:, tsl], in0=ps2,
                                             in1=th[h][:, tsl])

            # ---------- phase B: cross matmul + per-chunk argmax ----------
            with tc.tile_pool(name="cbf", bufs=3) as cbf_pool, \
                 tc.tile_pool(name="strips", bufs=4) as strips, \
                 tc.tile_pool(name="psB", bufs=5, space="PSUM") as psB:

                for cc in range(CCH):
                    csl = slice(cc * 512, (cc + 1) * 512)
                    cb_f = []
                    for h in range(HT):
                        t_ = cbf_pool.tile([128, 512], F16, name=f"cbf{h}",
                                           tag=f"cbf{h}")
                        nc.sync.dma_start(out=t_,
                                          in_=d_cb16[h * 128:(h + 1) * 128, csl])
                        cb_f.append(t_)
                    bias_cc = cbf_pool.tile([128, 512], F32, name="bias_cc",
                                            tag="bias_cc")
                    nc.sync.dma_start(
                        out=bias_cc,
                        in_=bass.AP(tensor=d_bias, offset=cc * 512,
                                    ap=[[0, 128], [1, 512]]))

                    for t in range(TT):
                        tsl = slice(t * 128, (t + 1) * 128)
                        acc = psB.tile([128, 512], F32, tag="acc")
                        for h in range(HT):
                            nc.tensor.matmul(acc, lhsT=th[h][:, tsl],
                                             rhs=cb_f[h], start=(h == 0),
                                             stop=False)
                        for h in range(HT):
                            nc.tensor.matmul(acc, lhsT=tl[h][:, tsl],
                                             rhs=cb_f[h], start=False,
                                             stop=(h == HT - 1))
                        s = strips.tile([128, 512], F32, tag="s")
                        nc.vector.tensor_sub(out=s, in0=acc, in1=bias_cc)
                        ssl = slice(cc * 8, (cc + 1) * 8)
                        nc.vector.max(out=gval[t][:, ssl], in_=s)
                        nc.vector.max_index(out=gidx[t][:, ssl],
                                            in_max=gval[t][:, ssl],
                                            in_values=s)

            # ---------- phase C: combine the 16 chunk winners ----------
            with tc.tile_pool(name="fin", bufs=2) as fin:
                for t in range(TT):
                    W = CCH * 8
                    gmx = fin.tile([128, 1], F32, tag="gmx")
                    nc.vector.tensor_reduce(out=gmx, in_=gval[t],
                                            axis=mybir.AxisListType.X,
                                            op=mybir.AluOpType.max)
                    mask = fin.tile([128, W], F32, tag="mask")
                    nc.vector.tensor_scalar(
                        out=mask, in0=gval[t], scalar1=gmx, scalar2=None,
                        op0=mybir.AluOpType.is_ge)
                    inv = fin.tile([128, W], F32, tag="inv")
                    nc.vector.tensor_scalar(
                        out=inv, in0=mask, scalar1=-16384.0, scalar2=16384.0,
                        op0=mybir.AluOpType.mult, op1=mybir.AluOpType.add)
                    cidxf = fin.tile([128, W], F32, tag="cidxf")
                    nc.vector.tensor_copy(cidxf, gidx[t])
                    cand = fin.tile([128, W], F32, tag="cand")
                    nc.vector.tensor_add(cand, cidxf, chunk_off)
                    nc.vector.tensor_add(cand, cand, inv)
                    win = fin.tile([128, 1], F32, tag="win")
                    nc.vector.tensor_reduce(out=win, in_=cand,
                                            axis=mybir.AxisListType.X,
                                            op=mybir.AluOpType.min)
                    lab = fin.tile([128, 1], I32, tag="lab")
                    nc.vector.tensor_copy(lab, win)
                    nc.sync.dma_start(out=d_lab[:, t:t + 1], in_=lab)

    nc.compile()
    return nc


_NC_CACHE = None


def make_in_maps(input_values, ln_weight, ln_bias, proj_weight, codebook):
    input_values = np.ascontiguousarray(input_values, np.float32)
    pwt = np.ascontiguousarray(proj_weight.astype(np.float32).T)
    lnw = np.ascontiguousarray(ln_weight, np.float32)
    lnb = np.ascontiguousarray(ln_bias, np.float32)
    cb16 = np.ascontiguousarray(codebook.astype(np.float16))
    bias = np.ascontiguousarray(
        (0.5 * SC * (codebook.astype(np.float64) ** 2).sum(0)).astype(np.float32))

    in_maps = []
    for i in range(N_CORES):
        in_maps.append({
            "x": np.ascontiguousarray(input_values[i]),
            "pwt": pwt, "lnw": lnw, "lnb": lnb, "cb16": cb16, "bias": bias,
        })
    return in_maps


def kernel(input_values, ln_weight, ln_bias, proj_weight, codebook):
    global _NC_CACHE
    if _NC_CACHE is None:
        _NC_CACHE = build_nc()
    nc = _NC_CACHE

    in_maps = make_in_maps(input_values, ln_weight, ln_bias, proj_weight,
                           codebook)
    res = run_bass_kernel_spmd(nc, in_maps, core_ids=list(range(N_CORES)))
    out = np.empty((B, L), np.int32)
    for i in range(N_CORES):
        out[i] = res.results[i]["labels"].T.reshape(L)
    return out


# revision 7
# speedup vs baseline: 1.6037x; 1.1323x over previous
"""Trainium2 Bass kernel for BestRQ vector-quantization codebook lookup.

Pipeline (per NeuronCore, data-parallel over batch):
  x (2048,512) --LayerNorm--> xn --PE transpose--> xnT (d-major)
  t^T = pwT^T @ xn^T  (fp32 matmul over d, exact)
  t split: th_s = fp16(t*2048) (hi, pre-scaled), tl = fp16((t*2048 - th_s))
  codebook pre-cast to fp16 on host; bias = 0.5*2048*||c||^2 precomputed
  on host (fp64) and broadcast-DMA'd per 512-col chunk.
  score*2048 = th_s@c16 + tl@c16   (2 fp16 passes, one PSUM group, fp32 acc)
  s = score*2048 - bias            (argmax == argmin of ||t-c||^2)
  per-chunk top8/max_index written into wide per-token-tile strips;
  global combine over 16 chunks picks the label.

Numerics: t is carried to ~22 fp16 bits (hi/lo), the codebook is single-
rounded fp16.  Measured against the fp64 oracle this flips 5 / 16384
labels (rel err 8.4e-3, gate is 2e-2).
"""

import numpy as np

import concourse.bacc as bacc
import concourse.bass as bass
import concourse.mybir as mybir
import concourse.tile as tile
from concourse.bass_utils import run_bass_kernel_spmd
from concourse.masks import make_identity

B, L, D, H, C = 8, 2048, 512, 1024, 8192
LN_EPS = 1e-5
N_CORES = 8

TT = L // 128      # 16 token tiles
CCH = C // 512     # 16 codebook chunks
HT = H // 128      # 8 h tiles
DT = D // 128      # 4 d tiles
TOKC = L // 512    # 4 token chunks (projection)
SC = 2048.0        # hi-part scale (exact power of two)

F32 = mybir.dt.float32
F16 = mybir.dt.float16
F8 = mybir.dt.float8e4
I32 = mybir.dt.int32
U32 = mybir.dt.uint32


def build_nc():
    nc = bacc.Bacc("TRN2", target_bir_lowering=False, debug=False)

    d_x = nc.dram_tensor("x", (L, D), F32, kind="ExternalInput")
    d_pwt = nc.dram_tensor("pwt", (D, H), F32, kind="ExternalInput")
    d_lnw = nc.dram_tensor("lnw", (D,), F32, kind="ExternalInput")
    d_lnb = nc.dram_tensor("lnb", (D,), F32, kind="ExternalInput")
    d_cb16 = nc.dram_tensor("cb16", (H, C), F16, kind="ExternalInput")
    d_bias = nc.dram_tensor("bias", (C,), F32, kind="ExternalInput")
    d_lab = nc.dram_tensor("labels", (128, TT), I32, kind="ExternalOutput")

    with tile.TileContext(nc) as tc:
        with tc.tile_pool(name="consts", bufs=1) as consts, \
             tc.tile_pool(name="persist", bufs=1) as persist:

            # ---------- constants ----------
            ident = consts.tile([128, 128], F32)
            make_identity(nc, ident)
            eps_t = consts.tile([128, 1], F32)
            nc.vector.memset(eps_t, LN_EPS)
            lnw_bc = consts.tile([128, D], F32)
            nc.sync.dma_start(
                out=lnw_bc,
                in_=bass.AP(tensor=d_lnw, offset=0, ap=[[0, 128], [1, D]]))
            lnb_bc = consts.tile([128, D], F32)
            nc.sync.dma_start(
                out=lnb_bc,
                in_=bass.AP(tensor=d_lnb, offset=0, ap=[[0, 128], [1, D]]))
            # per-slot chunk offset for the final combine: slot j -> 512*(j//8)
            chunk_off = consts.tile([128, CCH * 8], F32)
            for j in range(CCH):
                nc.vector.memset(chunk_off[:, j * 8:(j + 1) * 8], 512.0 * j)

            # persistent split of t^T: (h, tok) layout.  hi part fp16; lo
            # (residual) part fp8e4 in DoubleRow pair layout [128, 2, L]
            # (pair j holds h-tiles 2j / 2j+1 in slots 0 / 1).
            th = [persist.tile([128, L], F16, name=f"th{h}", tag=f"th{h}")
                  for h in range(HT)]
            tl8 = [persist.tile([128, 2, L], F8, name=f"tl8{j}", tag=f"tl8{j}")
                   for j in range(HT // 2)]
            # per-token-tile chunk winners: 8 slots per chunk (top8 desc)
            gval = [persist.tile([128, CCH * 8], F32, name=f"gval{t}",
                                 tag=f"gval{t}") for t in range(TT)]
            gidx = [persist.tile([128, CCH * 8], U32, name=f"gidx{t}",
                                 tag=f"gidx{t}") for t in range(TT)]

            # ---------- phase A: LN + transpose + projection + split ----------
            with tc.tile_pool(name="phA", bufs=1) as phA, \
                 tc.tile_pool(name="ldtmp", bufs=3) as ldtmp, \
                 tc.tile_pool(name="psA", bufs=4, space="PSUM") as psA, \
                 tc.tile_pool(name="psTr", bufs=2, space="PSUM") as psTr:

                # proj weight comes in pre-transposed: (d, h)
                pwT = [phA.tile([128, H], F32, name=f"pwT{d}", tag=f"pwT{d}")
                       for d in range(DT)]
                for d in range(DT):
                    nc.sync.dma_start(out=pwT[d],
                                      in_=d_pwt[d * 128:(d + 1) * 128, :])

                # LayerNorm + transpose to xnT (d, tok)
                xnT = [phA.tile([128, L], F32, name=f"xnT{d}", tag=f"xnT{d}")
                       for d in range(DT)]
                for t in range(TT):
                    x_t = ldtmp.tile([128, D], F32, tag="x_t")
                    nc.sync.dma_start(out=x_t, in_=d_x[t * 128:(t + 1) * 128, :])
                    stats = ldtmp.tile([128, 6], F32, tag="stats")
                    nc.vector.bn_stats(out=stats, in_=x_t)
                    mv = ldtmp.tile([128, 2], F32, tag="mv")
                    nc.vector.bn_aggr(out=mv, in_=stats)
                    rstd = ldtmp.tile([128, 1], F32, tag="rstd")
                    nc.scalar.activation(out=rstd, in_=mv[:, 1:2],
                                         func=mybir.ActivationFunctionType.Sqrt,
                                         bias=eps_t, scale=1.0)
                    nc.vector.reciprocal(out=rstd, in_=rstd)
                    xn = ldtmp.tile([128, D], F32, tag="xn")
                    nc.vector.tensor_scalar(
                        out=xn, in0=x_t, scalar1=mv[:, 0:1], scalar2=rstd,
                        op0=mybir.AluOpType.subtract, op1=mybir.AluOpType.mult)
                    nc.vector.tensor_mul(out=xn, in0=xn, in1=lnw_bc)
                    nc.vector.tensor_add(out=xn, in0=xn, in1=lnb_bc)
                    for d in range(DT):
                        ps_tr = psTr.tile([128, 128], F32, tag="ps_tr")
                        nc.tensor.transpose(ps_tr, xn[:, d * 128:(d + 1) * 128],
                                            ident)
                        nc.scalar.copy(out=xnT[d][:, t * 128:(t + 1) * 128],
                                       in_=ps_tr)

                # projection t^T[h, tok] = sum_d pwT[d][:,h].T @ xnT[d][:,tok]
                # fp32 (exact); split each PSUM into fp16 hi*2048 / fp16 lo.
                # tok-chunk outer so phase B can start on early token tiles.
                for tk in range(TOKC):
                    tsl = slice(tk * 512, (tk + 1) * 512)
                    for h in range(HT):
                        ps_t = psA.tile([128, 512], F32, tag="ps_t")
                        for d in range(DT):
                            nc.tensor.matmul(
                                ps_t,
                                lhsT=pwT[d][:, h * 128:(h + 1) * 128],
                                rhs=xnT[d][:, tsl],
                                start=(d == 0), stop=(d == DT - 1))
                        # th = fp16(t*2048)  (exact power-of-two scale)
                        nc.scalar.activation(out=th[h][:, tsl], in_=ps_t,
                                             func=mybir.ActivationFunctionType.Copy,
                                             scale=SC)
                        # tl8 = fp8e4(t*2048 - th): fp8 residual is plenty —
                        # it corrects at the 2^-11 level of an O(10) score.
                        ps2 = ldtmp.tile([128, 512], F32, tag="ps2")
                        nc.scalar.activation(out=ps2, in_=ps_t,
                                             func=mybir.ActivationFunctionType.Copy,
                                             scale=SC)
                        nc.vector.tensor_sub(
                            out=tl8[h // 2][:, h % 2, tsl], in0=ps2,
                            in1=th[h][:, tsl])

            # ---------- phase B: cross matmul + per-chunk argmax ----------
            with tc.tile_pool(name="cbf", bufs=3) as cbf_pool, \
                 tc.tile_pool(name="strips", bufs=4) as strips, \
                 tc.tile_pool(name="psB", bufs=5, space="PSUM") as psB:

                for cc in range(CCH):
                    csl = slice(cc * 512, (cc + 1) * 512)
                    cb_f = []
                    for h in range(HT):
                        t_ = cbf_pool.tile([128, 512], F16, name=f"cbf{h}",
                                           tag=f"cbf{h}")
                        nc.sync.dma_start(out=t_,
                                          in_=d_cb16[h * 128:(h + 1) * 128, csl])
                        cb_f.append(t_)
                    # fp8 copy of the chunk in DoubleRow pair layout
                    cb8 = []
                    for j in range(HT // 2):
                        t8 = cbf_pool.tile([128, 2, 512], F8, name=f"cb8{j}",
                                           tag=f"cb8{j}")
                        nc.scalar.copy(out=t8[:, 0, :], in_=cb_f[2 * j])
                        nc.scalar.copy(out=t8[:, 1, :], in_=cb_f[2 * j + 1])
                        cb8.append(t8)
                    bias_cc = cbf_pool.tile([128, 512], F32, name="bias_cc",
                                            tag="bias_cc")
                    nc.sync.dma_start(
                        out=bias_cc,
                        in_=bass.AP(tensor=d_bias, offset=cc * 512,
                                    ap=[[0, 128], [1, 512]]))

                    for t in range(TT):
                        tsl = slice(t * 128, (t + 1) * 128)
                        acc = psB.tile([128, 512], F32, tag="acc")
                        for h in range(HT):
                            nc.tensor.matmul(acc, lhsT=th[h][:, tsl],
                                             rhs=cb_f[h], start=(h == 0),
                                             stop=False)
                        for j in range(HT // 2):
                            nc.tensor.matmul(
                                acc, lhsT=tl8[j][:, :, tsl], rhs=cb8[j],
                                perf_mode=mybir.MatmulPerfMode.DoubleRow,
                                start=False, stop=(j == HT // 2 - 1))
                        s = strips.tile([128, 512], F32, tag="s")
                        nc.vector.tensor_sub(out=s, in0=acc, in1=bias_cc)
                        ssl = slice(cc * 8, (cc + 1) * 8)
                        nc.vector.max(out=gval[t][:, ssl], in_=s)
                        nc.vector.max_index(out=gidx[t][:, ssl],
                                            in_max=gval[t][:, ssl],
                                            in_values=s)

            # ---------- phase C: combine the 16 chunk winners ----------
            with tc.tile_pool(name="fin", bufs=2) as fin:
                for t in range(TT):
                    W = CCH * 8
                    gmx = fin.tile([128, 1], F32, tag="gmx")
                    nc.vector.tensor_reduce(out=gmx, in_=gval[t],
                                            axis=mybir.AxisListType.X,
                                            op=mybir.AluOpType.max)
                    mask = fin.tile([128, W], F32, tag="mask")
                    nc.vector.tensor_scalar(
                        out=mask, in0=gval[t], scalar1=gmx, scalar2=None,
                        op0=mybir.AluOpType.is_ge)
                    inv = fin.tile([128, W], F32, tag="inv")
                    nc.vector.tensor_scalar(
                        out=inv, in0=mask, scalar1=-16384.0, scalar2=16384.0,
                        op0=mybir.AluOpType.mult, op1=mybir.AluOpType.add)
                    cidxf = fin.tile([128, W], F32, tag="cidxf")
                    nc.vector.tensor_copy(cidxf, gidx[t])
                    cand = fin.tile([128, W], F32, tag="cand")
                    nc.vector.tensor_add(cand, cidxf, chunk_off)
                    nc.vector.tensor_add(cand, cand, inv)
                    win = fin.tile([128, 1], F32, tag="win")
                    nc.vector.tensor_reduce(out=win, in_=cand,
                                            axis=mybir.AxisListType.X,
                                            op=mybir.AluOpType.min)
                    lab = fin.tile([128, 1], I32, tag="lab")
                    nc.vector.tensor_copy(lab, win)
                    nc.sync.dma_start(out=d_lab[:, t:t + 1], in_=lab)

    nc.compile()
    return nc


_NC_CACHE = None


def make_in_maps(input_values, ln_weight, ln_bias, proj_weight, codebook):
    input_values = np.ascontiguousarray(input_values, np.float32)
    pwt = np.ascontiguousarray(proj_weight.astype(np.float32).T)
    lnw = np.ascontiguousarray(ln_weight, np.float32)
    lnb = np.ascontiguousarray(ln_bias, np.float32)
    cb16 = np.ascontiguousarray(codebook.astype(np.float16))
    bias = np.ascontiguousarray(
        (0.5 * SC * (codebook.astype(np.float64) ** 2).sum(0)).astype(np.float32))

    in_maps = []
    for i in range(N_CORES):
        in_maps.append({
            "x": np.ascontiguousarray(input_values[i]),
            "pwt": pwt, "lnw": lnw, "lnb": lnb, "cb16": cb16, "bias": bias,
        })
    return in_maps


def kernel(input_values, ln_weight, ln_bias, proj_weight, codebook):
    global _NC_CACHE
    if _NC_CACHE is None:
        _NC_CACHE = build_nc()
    nc = _NC_CACHE

    in_maps = make_in_maps(input_values, ln_weight, ln_bias, proj_weight,
                           codebook)
    res = run_bass_kernel_spmd(nc, in_maps, core_ids=list(range(N_CORES)))
    out = np.empty((B, L), np.int32)
    for i in range(N_CORES):
        out[i] = res.results[i]["labels"].T.reshape(L)
    return out


# revision 20
# speedup vs baseline: 1.7474x; 1.0896x over previous
"""Trainium2 Bass kernel for BestRQ vector-quantization codebook lookup.

Pipeline (per NeuronCore, data-parallel over batch):
  x (2048,512) --LayerNorm--> xn --PE transpose--> xnT (d-major)
  t^T = pwT^T @ xn^T  (fp32 matmul over d, exact)
  t split: th_s = fp16(t*2048) (hi, pre-scaled), tl = fp16((t*2048 - th_s))
  codebook pre-cast to fp16 on host; bias = 0.5*2048*||c||^2 precomputed
  on host (fp64) and broadcast-DMA'd per 512-col chunk.
  score*2048 = th_s@c16 + tl@c16   (2 fp16 passes, one PSUM group, fp32 acc)
  s = score*2048 - bias            (argmax == argmin of ||t-c||^2)
  per-chunk top8/max_index written into wide per-token-tile strips;
  global combine over 16 chunks picks the label.

Numerics: t is carried to ~22 fp16 bits (hi/lo), the codebook is single-
rounded fp16.  Measured against the fp64 oracle this flips 5 / 16384
labels (rel err 8.4e-3, gate is 2e-2).
"""

import numpy as np

import concourse.bacc as bacc
import concourse.bass as bass
import concourse.mybir as mybir
import concourse.tile as tile
from concourse.bass_utils import run_bass_kernel_spmd
from concourse.masks import make_identity

B, L, D, H, C = 8, 2048, 512, 1024, 8192
LN_EPS = 1e-5
N_CORES = 8

TT = L // 128      # 16 token tiles
CCH = C // 512     # 16 codebook chunks
HT = H // 128      # 8 h tiles
DT = D // 128      # 4 d tiles
TOKC = L // 512    # 4 token chunks (projection)
SC = 2048.0        # hi-part scale (exact power of two)

F32 = mybir.dt.float32
F16 = mybir.dt.float16
F8 = mybir.dt.float8e4
I32 = mybir.dt.int32
U32 = mybir.dt.uint32

# packed-input element offsets
OFF_X = 0                      # (L, D) f32
OFF_LNW = OFF_X + L * D        # (D,) f32
OFF_LNB = OFF_LNW + D          # (D,) f32
OFF_BIAS = OFF_LNB + D         # (C,) f32
NF32 = OFF_BIAS + C
OFF_PH = 0                     # (D, H) f16
OFF_PL = OFF_PH + D * H        # (D, H) f16
OFF_CB = OFF_PL + D * H        # (H, C) f16
NF16 = OFF_CB + H * C


def build_nc():
    nc = bacc.Bacc("TRN2", target_bir_lowering=False, debug=False)

    # inputs are packed into two flat tensors (one per dtype): each extra
    # PJRT argument costs ~56us of per-call dispatch overhead on this setup.
    d_f32 = nc.dram_tensor("in32", (NF32,), F32, kind="ExternalInput")
    d_f16 = nc.dram_tensor("in16", (NF16,), F16, kind="ExternalInput")
    d_lab = nc.dram_tensor("labels", (128, TT), I32, kind="ExternalOutput")

    with tile.TileContext(nc) as tc:
        with tc.tile_pool(name="consts", bufs=1) as consts, \
             tc.tile_pool(name="persist", bufs=1) as persist:

            # ---------- constants ----------
            ident = consts.tile([128, 128], F32)
            make_identity(nc, ident)
            eps_t = consts.tile([128, 1], F32)
            nc.vector.memset(eps_t, LN_EPS)
            lnw_bc = consts.tile([128, D], F32)
            nc.sync.dma_start(
                out=lnw_bc,
                in_=bass.AP(tensor=d_f32, offset=OFF_LNW, ap=[[0, 128], [1, D]]))
            lnb_bc = consts.tile([128, D], F32)
            nc.sync.dma_start(
                out=lnb_bc,
                in_=bass.AP(tensor=d_f32, offset=OFF_LNB, ap=[[0, 128], [1, D]]))
            # per-slot chunk offset for the final combine: slot j -> 512*(j//8)
            chunk_off = consts.tile([128, CCH * 8], F32)
            for j in range(CCH):
                nc.vector.memset(chunk_off[:, j * 8:(j + 1) * 8], 512.0 * j)

            # persistent split of t^T: (h, tok) layout.  hi part fp16; lo
            # (residual) part fp8e4 in DoubleRow pair layout [128, 2, L]
            # (pair j holds h-tiles 2j / 2j+1 in slots 0 / 1).
            th = [persist.tile([128, L], F16, name=f"th{h}", tag=f"th{h}")
                  for h in range(HT)]
            tl8 = [persist.tile([128, 2, L], F8, name=f"tl8{j}", tag=f"tl8{j}")
                   for j in range(HT // 2)]
            # per-token-tile chunk winners: 8 slots per chunk (top8 desc)
            gval = [persist.tile([128, CCH * 8], F32, name=f"gval{t}",
                                 tag=f"gval{t}") for t in range(TT)]
            gidx = [persist.tile([128, CCH * 8], U32, name=f"gidx{t}",
                                 tag=f"gidx{t}") for t in range(TT)]

            # ---------- phase A: LN + transpose + projection + split ----------
            with tc.tile_pool(name="phA", bufs=1) as phA, \
                 tc.tile_pool(name="ldtmp", bufs=3) as ldtmp, \
                 tc.tile_pool(name="psA", bufs=4, space="PSUM") as psA, \
                 tc.tile_pool(name="psTr", bufs=2, space="PSUM") as psTr:

                # proj weight comes in pre-transposed (d, h) and pre-split on
                # host: ph = fp16(pwT), pl = fp16((pwT - ph) * 2048).
                # p8 pairs (fp8) feed the DoubleRow cross-term matmuls:
                # slot 0 = fp8(ph) (x lo), slot 1 = fp8(pl) (x hi).
                ph16 = [phA.tile([128, H], F16, name=f"ph16{d}", tag=f"ph16{d}")
                        for d in range(DT)]
                p8 = [phA.tile([128, 2, H], F8, name=f"p8{d}", tag=f"p8{d}")
                      for d in range(DT)]
                for d in range(DT):
                    nc.sync.dma_start(
                        out=ph16[d],
                        in_=bass.AP(tensor=d_f16, offset=OFF_PH + d * 128 * H,
                                    ap=[[H, 128], [1, H]]))
                    pl_t = ldtmp.tile([128, H], F16, tag="pl_t")
                    nc.sync.dma_start(
                        out=pl_t,
                        in_=bass.AP(tensor=d_f16, offset=OFF_PL + d * 128 * H,
                                    ap=[[H, 128], [1, H]]))
                    nc.scalar.copy(out=p8[d][:, 0, :], in_=ph16[d])
                    nc.scalar.copy(out=p8[d][:, 1, :], in_=pl_t)

                # LayerNorm + transpose to xnT (d, tok)
                xnT = [phA.tile([128, L], F32, name=f"xnT{d}", tag=f"xnT{d}")
                       for d in range(DT)]
                # fp16/fp8 split of xnT: xh2048 = fp16(xn*2048); x8 pairs
                # hold fp8 lo (slot 0, pairs with fp8 ph) / fp8 hi (slot 1,
                # pairs with fp8 pl), all at the 2048 scale convention.
                xh2048 = [phA.tile([128, L], F16, name=f"xh{d}", tag=f"xh{d}")
                          for d in range(DT)]
                x8 = [phA.tile([128, 2, L], F8, name=f"x8{d}", tag=f"x8{d}")
                      for d in range(DT)]
                for t in range(TT):
                    x_t = ldtmp.tile([128, D], F32, tag="x_t")
                    nc.sync.dma_start(
                        out=x_t,
                        in_=bass.AP(tensor=d_f32, offset=OFF_X + t * 128 * D,
                                    ap=[[D, 128], [1, D]]))
                    stats = ldtmp.tile([128, 6], F32, tag="stats")
                    nc.vector.bn_stats(out=stats, in_=x_t)
                    mv = ldtmp.tile([128, 2], F32, tag="mv")
                    nc.vector.bn_aggr(out=mv, in_=stats)
                    rstd = ldtmp.tile([128, 1], F32, tag="rstd")
                    nc.scalar.activation(out=rstd, in_=mv[:, 1:2],
                                         func=mybir.ActivationFunctionType.Sqrt,
                                         bias=eps_t, scale=1.0)
                    nc.vector.reciprocal(out=rstd, in_=rstd)
                    xn = ldtmp.tile([128, D], F32, tag="xn")
                    nc.vector.tensor_scalar(
                        out=xn, in0=x_t, scalar1=mv[:, 0:1], scalar2=rstd,
                        op0=mybir.AluOpType.subtract, op1=mybir.AluOpType.mult)
                    nc.vector.tensor_mul(out=xn, in0=xn, in1=lnw_bc)
                    nc.vector.tensor_add(out=xn, in0=xn, in1=lnb_bc)
                    for d in range(DT):
                        ps_tr = psTr.tile([128, 128], F32, tag="ps_tr")
                        nc.tensor.transpose(ps_tr, xn[:, d * 128:(d + 1) * 128],
                                            ident)
                        nc.scalar.copy(out=xnT[d][:, t * 128:(t + 1) * 128],
                                       in_=ps_tr)

                # projection t^T[h, tok] = sum_d pwT[d][:,h].T @ xnT[d][:,tok]
                # computed at the 2048 scale in one PSUM group per (tk, h):
                #   ph16 @ xh2048            (fp16 hi x hi, 4 matmuls)
                #   fp8(ph) (x) fp8(x lo) + fp8(pl) (x) fp8(x hi)
                #                            (DoubleRow cross terms, 4 matmuls)
                # (pl (x) x-lo term is 2^-22: dropped.)
                # tok-chunk outer so phase B can start on early token tiles.
                for tk in range(TOKC):
                    tsl = slice(tk * 512, (tk + 1) * 512)
                    for d in range(DT):
                        nc.scalar.activation(out=xh2048[d][:, tsl],
                                             in_=xnT[d][:, tsl],
                                             func=mybir.ActivationFunctionType.Copy,
                                             scale=SC)
                        nc.scalar.activation(out=x8[d][:, 1, tsl],
                                             in_=xh2048[d][:, tsl],
                                             func=mybir.ActivationFunctionType.Copy,
                                             scale=1.0 / SC)
                        xn2048 = ldtmp.tile([128, 512], F32, tag="xn2048")
                        nc.scalar.activation(out=xn2048, in_=xnT[d][:, tsl],
                                             func=mybir.ActivationFunctionType.Copy,
                                             scale=SC)
                        nc.vector.tensor_sub(out=x8[d][:, 0, tsl], in0=xn2048,
                                             in1=xh2048[d][:, tsl])
                    for h in range(HT):
                        hsl = slice(h * 128, (h + 1) * 128)
                        ps_t = psA.tile([128, 512], F32, tag="ps_t")
                        for d in range(DT):
                            nc.tensor.matmul(
                                ps_t, lhsT=ph16[d][:, hsl],
                                rhs=xh2048[d][:, tsl],
                                start=(d == 0), stop=False)
                        for d in range(DT):
                            nc.tensor.matmul(
                                ps_t, lhsT=p8[d][:, :, hsl],
                                rhs=x8[d][:, :, tsl],
                                perf_mode=mybir.MatmulPerfMode.DoubleRow,
                                start=False, stop=(d == DT - 1))
                        # ps_t is already t*2048: split directly off PSUM.
                        nc.scalar.copy(out=th[h][:, tsl], in_=ps_t)
                        nc.vector.tensor_sub(
                            out=tl8[h // 2][:, h % 2, tsl], in0=ps_t,
                            in1=th[h][:, tsl])

            # ---------- phase B: cross matmul + per-chunk argmax ----------
            with tc.tile_pool(name="cbf", bufs=3) as cbf_pool, \
                 tc.tile_pool(name="strips", bufs=4) as strips, \
                 tc.tile_pool(name="psB", bufs=5, space="PSUM") as psB:

                for cc in range(CCH):
                    csl = slice(cc * 512, (cc + 1) * 512)
                    cb_f = []
                    for h in range(HT):
                        t_ = cbf_pool.tile([128, 512], F16, name=f"cbf{h}",
                                           tag=f"cbf{h}")
                        nc.sync.dma_start(
                            out=t_,
                            in_=bass.AP(tensor=d_f16,
                                        offset=OFF_CB + h * 128 * C + cc * 512,
                                        ap=[[C, 128], [1, 512]]))
                        cb_f.append(t_)
                    # fp8 copy of the chunk in DoubleRow pair layout
                    cb8 = []
                    for j in range(HT // 2):
                        t8 = cbf_pool.tile([128, 2, 512], F8, name=f"cb8{j}",
                                           tag=f"cb8{j}")
                        nc.scalar.copy(out=t8[:, 0, :], in_=cb_f[2 * j])
                        nc.scalar.copy(out=t8[:, 1, :], in_=cb_f[2 * j + 1])
                        cb8.append(t8)
                    bias_cc = cbf_pool.tile([128, 512], F32, name="bias_cc",
                                            tag="bias_cc")
                    nc.sync.dma_start(
                        out=bias_cc,
                        in_=bass.AP(tensor=d_f32, offset=OFF_BIAS + cc * 512,
                                    ap=[[0, 128], [1, 512]]))

                    for t in range(TT):
                        tsl = slice(t * 128, (t + 1) * 128)
                        acc = psB.tile([128, 512], F32, tag="acc")
                        for h in range(HT):
                            nc.tensor.matmul(acc, lhsT=th[h][:, tsl],
                                             rhs=cb_f[h], start=(h == 0),
                                             stop=False)
                        for j in range(HT // 2):
                            nc.tensor.matmul(
                                acc, lhsT=tl8[j][:, :, tsl], rhs=cb8[j],
                                perf_mode=mybir.MatmulPerfMode.DoubleRow,
                                start=False, stop=(j == HT // 2 - 1))
                        s = strips.tile([128, 512], F32, tag="s")
                        nc.vector.tensor_sub(out=s, in0=acc, in1=bias_cc)
                        ssl = slice(cc * 8, (cc + 1) * 8)
                        nc.vector.max(out=gval[t][:, ssl], in_=s)
                        nc.vector.max_index(out=gidx[t][:, ssl],
                                            in_max=gval[t][:, ssl],
                                            in_values=s)

            # ---------- phase C: combine the 16 chunk winners ----------
            with tc.tile_pool(name="fin", bufs=2) as fin:
                for t in range(TT):
                    W = CCH * 8
                    gmx = fin.tile([128, 1], F32, tag="gmx")
                    nc.vector.tensor_reduce(out=gmx, in_=gval[t],
                                            axis=mybir.AxisListType.X,
                                            op=mybir.AluOpType.max)
                    mask = fin.tile([128, W], F32, tag="mask")
                    nc.vector.tensor_scalar(
                        out=mask, in0=gval[t], scalar1=gmx, scalar2=None,
                        op0=mybir.AluOpType.is_ge)
                    inv = fin.tile([128, W], F32, tag="inv")
                    nc.vector.tensor_scalar(
                        out=inv, in0=mask, scalar1=-16384.0, scalar2=16384.0,
                        op0=mybir.AluOpType.mult, op1=mybir.AluOpType.add)
                    cidxf = fin.tile([128, W], F32, tag="cidxf")
                    nc.vector.tensor_copy(cidxf, gidx[t])
                    cand = fin.tile([128, W], F32, tag="cand")
                    nc.vector.tensor_add(cand, cidxf, chunk_off)
                    nc.vector.tensor_add(cand, cand, inv)
                    win = fin.tile([128, 1], F32, tag="win")
                    nc.vector.tensor_reduce(out=win, in_=cand,
                                            axis=mybir.AxisListType.X,
                                            op=mybir.AluOpType.min)
                    lab = fin.tile([128, 1], I32, tag="lab")
                    nc.vector.tensor_copy(lab, win)
                    nc.sync.dma_start(out=d_lab[:, t:t + 1], in_=lab)

    nc.compile()
    return nc


_NC_CACHE = None


def make_in_maps(input_values, ln_weight, ln_bias, proj_weight, codebook):
    input_values = np.ascontiguousarray(input_values, np.float32)
    pwt = proj_weight.astype(np.float32).T
    ph = pwt.astype(np.float16)
    pl = ((pwt - ph.astype(np.float32)) * SC).astype(np.float16)
    lnw = np.asarray(ln_weight, np.float32)
    lnb = np.asarray(ln_bias, np.float32)
    cb16 = codebook.astype(np.float16)
    bias = (0.5 * SC * (codebook.astype(np.float64) ** 2).sum(0)).astype(
        np.float32)

    in16 = np.concatenate([ph.ravel(), pl.ravel(), cb16.ravel()])
    in16 = np.ascontiguousarray(in16, np.float16)

    in_maps = []
    for i in range(N_CORES):
        in32 = np.concatenate([input_values[i].ravel(), lnw.ravel(),
                               lnb.ravel(), bias.ravel()])
        in_maps.append({
            "in32": np.ascontiguousarray(in32, np.float32),
            "in16": in16,
        })
    return in_maps


def kernel(input_values, ln_weight, ln_bias, proj_weight, codebook):
    global _NC_CACHE
    if _NC_CACHE is None:
        _NC_CACHE = build_nc()
    nc = _NC_CACHE

    in_maps = make_in_maps(input_values, ln_weight, ln_bias, proj_weight,
                           codebook)
    res = run_bass_kernel_spmd(nc, in_maps, core_ids=list(range(N_CORES)))
    out = np.empty((B, L), np.int32)
    for i in range(N_CORES):
        out[i] = res.results[i]["labels"].T.reshape(L)
    return out
